# revision 40
# baseline (speedup 1.0000x reference)
"""Causal multi-head attention (ChunkedDotProdAttention) on 8 TRN2 NeuronCores.

Problem: q,k,v [2, 2048, 2048] f32, 16 heads of dh=128, causal mask
(masked scores set to -50000 -> softmax -> exactly 0 in f32), out = attn @ v.

Sharding: 32 (batch, head) pairs, 4 per core; each core computes full
attention for its pairs — no cross-device comm.

Per-core kernel layout trick: everything is computed transposed.
  - host pre-transposes q,k to [dh, n] (bf16) so no on-chip input transposes;
    v is host-packed to the exact SBUF layout for a flat contiguous DMA
  - S^T[k, q] = K_j^T.T @ Q^T per (key-block j, 1024-query chunk c), causal
    blocks only, narrowed to valid queries; one [128,1024] psum S tile per
    key block so the exp runs once per block (amortizes ACT access latency)
  - P^T = exp(scale * S^T) on ACT (no max subtraction needed: |scale*s| < ~10)
  - diagonal blocks: triangular zero via gpsimd affine_select
  - out^T[d, q] += V_j.T @ P^T_j accumulated in PSUM (V_j natural layout!)
  - softmax denominators: running bf16 accumulator on DVE (spread across the
    chunk, narrowed, no tail-heavy tree) + a tiny ones-matmul on PE
  - normalize: reciprocal_approx_fast + gpsimd partition_broadcast + DVE mul
  - host post-transposes out^T back to natural layout
PE work = QK + PV streaming only ~= compute roofline; ACT exp is the
co-bottleneck (1 elem/cycle/lane is an ACT hardware floor).
"""

import numpy as np
import ml_dtypes

B = 2
N = 2048
D_MODEL = 2048
H = 16
DH = 128
N_CORES = 8
PAIRS_PER_CORE = (B * H) // N_CORES  # 4
SCALE = float(DH) ** -0.5
CHUNK = 1024  # query chunk (2 psum banks; one key-block row per S tile)
NCHUNKS = N // CHUNK  # 2
QB = 128  # query/key block
NB = N // QB  # 16 key blocks
BLOCKS_PER_CHUNK = CHUNK // QB  # 8
MMN = 512  # max matmul free dim (one psum bank of f32)

_nc_cache = {}
_last_in_maps = None


def _build_nc(reps=0):
    """Build + compile the per-core Bass kernel (same NEFF for all cores).

    reps>0 wraps the body in a dynamic For_i loop running it `reps` times —
    used only for wall-clock benchmarking (the work is idempotent)."""
    from contextlib import ExitStack

    import concourse.bass as bass
    import concourse.mybir as mybir
    import concourse.tile as tile
    from concourse import bacc
    from concourse import bass_isa

    dt_mm = mybir.dt.bfloat16
    f32 = mybir.dt.float32

    nc = bacc.Bacc(
        "TRN2",
        target_bir_lowering=False,
        debug=False,
        enable_asserts=False,
        num_devices=N_CORES,
    )
    P = PAIRS_PER_CORE
    qT_d = nc.dram_tensor("qT", [P, DH, N], dt_mm, kind="ExternalInput").ap()
    kT_d = nc.dram_tensor("kT", [P, DH, N], dt_mm, kind="ExternalInput").ap()
    # v pre-arranged on host to the SBUF layout: [pair, k_local, block*DH+d]
    v_d = nc.dram_tensor("v", [P, QB, NB * DH], dt_mm, kind="ExternalInput").ap()
    outT_d = nc.dram_tensor("outT", [P, DH, N], f32, kind="ExternalOutput").ap()
    # raw softmax-denominator accumulators; the host does the partition-sum
    # and the division (kills the PAR->recip->mul tail chain on device)
    accs_d = nc.dram_tensor(
        "accs", [P, NCHUNKS, QB, CHUNK], dt_mm, kind="ExternalOutput"
    ).ap()

    with tile.TileContext(nc) as tc, ExitStack() as ctx:
        sb = ctx.enter_context(tc.tile_pool(name="sb", bufs=3))
        pt_pool = ctx.enter_context(tc.tile_pool(name="pt", bufs=3))
        acc_pool = ctx.enter_context(tc.tile_pool(name="acc", bufs=2))
        small = ctx.enter_context(tc.tile_pool(name="small", bufs=3))
        outp = ctx.enter_context(tc.tile_pool(name="outp", bufs=2))
        const_pool = ctx.enter_context(tc.tile_pool(name="const", bufs=1))
        ps_s = ctx.enter_context(tc.tile_pool(name="ps_s", bufs=2, space="PSUM"))
        ps_o = ctx.enter_context(tc.tile_pool(name="ps_o", bufs=2, space="PSUM"))

        rep_ctx = ExitStack()
        if reps:
            # hint_engines: body >256 instrs/engine -> back-edge would
            # IRAM-miss (~4us) without prefetch hints; keeps the bench
            # closer to true single-shot time
            rep_ctx.enter_context(
                tc.For_i(
                    0,
                    reps,
                    1,
                    hint_engines=(
                        mybir.EngineType.PE,
                        mybir.EngineType.Activation,
                        mybir.EngineType.DVE,
                        mybir.EngineType.Pool,
                        mybir.EngineType.SP,
                    ),
                )
            )

        def emit_block(st, c, j, jc):
            qoff = max(0, j * QB - c * CHUNK)
            # split [qoff, CHUNK) into <=MMN psum-bank-aligned pieces
            pieces = [(qoff, MMN), (MMN, CHUNK)] if qoff < MMN else [(qoff, CHUNK)]
            s_ps = ps_s.tile([128, CHUNK], f32, tag="s")
            # hoist QKs ahead of the previous block's exp-gated PV cluster
            # (and the previous chunk/pair tail) in the static schedule
            with tc.high_priority(offset=40 if j < 2 else 12):
                for p0, p1 in pieces:
                    nc.tensor.matmul(
                        s_ps[:, p0:p1],
                        lhsT=st["kT"][:, j * QB : (j + 1) * QB],
                        rhs=st["qT"][:, c * CHUNK + p0 : c * CHUNK + p1],
                        start=True,
                        stop=True,
                    )
            pt = st["pt"]
            nc.scalar.activation(
                pt[:, j * CHUNK + qoff : (j + 1) * CHUNK],
                s_ps[:, qoff:],
                mybir.ActivationFunctionType.Exp,
                scale=SCALE,
            )
            if j * QB >= c * CHUNK:  # diagonal block
                # strict-upper triangle of the first 128 valid cols:
                # keep where local_q - local_k >= 0
                nc.gpsimd.affine_select(
                    out=pt[:, j * CHUNK + qoff : j * CHUNK + qoff + QB],
                    in_=pt[:, j * CHUNK + qoff : j * CHUNK + qoff + QB],
                    compare_op=mybir.AluOpType.is_ge,
                    fill=0.0,
                    base=0,
                    channel_multiplier=-1,
                    pattern=[[1, QB]],
                )
            for p0, p1 in pieces:
                nc.tensor.matmul(
                    st["o"][:, p0:p1],
                    lhsT=st["v"][:, j * DH : (j + 1) * DH],
                    rhs=pt[:, j * CHUNK + p0 : j * CHUNK + p1],
                    start=(j == 0),
                    stop=(j == jc - 1),
                )
            # running denominator accumulation (narrowed; spread across
            # the chunk instead of a tail-heavy tree fold)
            if j == 0:
                nc.vector.tensor_copy(st["acc"][:], pt[:, 0:CHUNK])
            else:
                nc.vector.tensor_add(
                    st["acc"][:, qoff:],
                    st["acc"][:, qoff:],
                    pt[:, j * CHUNK + qoff : (j + 1) * CHUNK],
                )

        def emit_tail(st, c):
            # unnormalized out^T to SBUF; acc straight out to DRAM — the
            # host finishes the softmax division
            nc.vector.tensor_copy(
                st["outT"][:, c * CHUNK : (c + 1) * CHUNK], st["o"][:]
            )
            nc.sync.dma_start(accs_d[st["p"], c], st["acc"][:])

        for p in range(P):
            # split input DMAs so the first QK blocks start before the
            # whole pair's operands land
            qT_s = sb.tile([128, N], dt_mm, tag="qT")
            nc.sync.dma_start(qT_s[:, :CHUNK], qT_d[p][:, :CHUNK])
            nc.sync.dma_start(qT_s[:, CHUNK:], qT_d[p][:, CHUNK:])
            kT_s = sb.tile([128, N], dt_mm, tag="kT")
            nc.sync.dma_start(kT_s[:, : 4 * QB], kT_d[p][:, : 4 * QB])
            nc.sync.dma_start(kT_s[:, 4 * QB :], kT_d[p][:, 4 * QB :])
            v_s = sb.tile([128, NB * DH], dt_mm, tag="v")
            nc.sync.dma_start(v_s[:, : 4 * DH], v_d[p][:, : 4 * DH])
            nc.sync.dma_start(v_s[:, 4 * DH :], v_d[p][:, 4 * DH :])
            outT_s = outp.tile([128, N], f32, tag="outT")
            st = {"p": p, "qT": qT_s, "kT": kT_s, "v": v_s, "outT": outT_s}
            for c in range(NCHUNKS):
                jc = BLOCKS_PER_CHUNK * (c + 1)  # key blocks 0..jc-1
                st["pt"] = pt_pool.tile([128, NB * CHUNK], dt_mm, tag="pt", name="pt")
                st["acc"] = acc_pool.tile([128, CHUNK], dt_mm, tag="acc", name="acc")
                st["o"] = ps_o.tile([128, CHUNK], f32, tag="o", name="o")
                for j in range(jc):
                    emit_block(st, c, j, jc)
                emit_tail(st, c)
            nc.sync.dma_start(outT_d[p], outT_s[:])

        rep_ctx.close()

    nc.compile()
    return nc


def _get_nc():
    if "nc" not in _nc_cache:
        _nc_cache["nc"] = _build_nc()
    return _nc_cache["nc"]


def kernel(q, k, v):
    from concourse.bass_utils import run_bass_kernel_spmd

    q = np.asarray(q, dtype=np.float32)
    k = np.asarray(k, dtype=np.float32)
    v = np.asarray(v, dtype=np.float32)

    bf16 = ml_dtypes.bfloat16
    # [b, n, h, dh] -> [b, h, dh, n] for q/k; [b, h, n, dh] for v
    qT = np.ascontiguousarray(
        q.reshape(B, N, H, DH).transpose(0, 2, 3, 1)
    ).astype(bf16)
    kT = np.ascontiguousarray(
        k.reshape(B, N, H, DH).transpose(0, 2, 3, 1)
    ).astype(bf16)
    # v -> [b, h, k_local, block, dh]: v_host[p, k, j*DH+d] = v[p, j*QB+k, d]
    vh = np.ascontiguousarray(
        v.reshape(B, NB, QB, H, DH).transpose(0, 3, 2, 1, 4)
    ).astype(bf16)

    qT = qT.reshape(B * H, DH, N)
    kT = kT.reshape(B * H, DH, N)
    vh = vh.reshape(B * H, QB, NB * DH)

    in_maps = []
    for core in range(N_CORES):
        lo = core * PAIRS_PER_CORE
        hi = lo + PAIRS_PER_CORE
        in_maps.append(
            {
                "qT": np.ascontiguousarray(qT[lo:hi]),
                "kT": np.ascontiguousarray(kT[lo:hi]),
                "v": np.ascontiguousarray(vh[lo:hi]),
            }
        )

    global _last_in_maps
    _last_in_maps = in_maps

    nc = _get_nc()
    res = run_bass_kernel_spmd(nc, in_maps, core_ids=list(range(N_CORES)))

    # reassemble: outT per core [P, dh, n] f32 (unnormalized) -> normalize by
    # the softmax denominators (sum the raw accumulators over partitions),
    # then back to [b, n, h*dh]
    outT = np.concatenate([r["outT"] for r in res.results], axis=0)  # [32, dh, n]
    accs = np.concatenate([r["accs"] for r in res.results], axis=0)
    # accs: [32, NCHUNKS, 128, CHUNK] bf16 -> sums [32, n]
    sums = accs.astype(np.float32).sum(axis=2).reshape(B * H, N)
    outT = outT / sums[:, None, :]
    out = outT.reshape(B, H, DH, N).transpose(0, 3, 1, 2).reshape(B, N, D_MODEL)
    return np.ascontiguousarray(out)


# revision 42
# speedup vs baseline: 1.7548x; 1.7548x over previous
"""Causal multi-head attention (ChunkedDotProdAttention) on 8 TRN2 NeuronCores.

Problem: q,k,v [2, 2048, 2048] f32, 16 heads of dh=128, causal mask
(masked scores set to -50000 -> softmax -> exactly 0 in f32), out = attn @ v.

Sharding: 32 (batch, head) pairs, 4 per core; each core computes full
attention for its pairs — no cross-device comm.

Per-core kernel layout trick: everything is computed transposed.
  - host pre-transposes q,k to [dh, n] (bf16) so no on-chip input transposes;
    v is host-packed to the exact SBUF layout for a flat contiguous DMA
  - S^T[k, q] = K_j^T.T @ Q^T per (key-block j, 1024-query chunk c), causal
    blocks only, narrowed to valid queries; one [128,1024] psum S tile per
    key block so the exp runs once per block (amortizes ACT access latency)
  - P^T = exp(scale * S^T) on ACT (no max subtraction needed: |scale*s| < ~10)
  - diagonal blocks: triangular zero via gpsimd affine_select
  - out^T[d, q] += V_j.T @ P^T_j accumulated in PSUM (V_j natural layout!)
  - softmax denominators: running bf16 accumulator on DVE (spread across the
    chunk, narrowed, no tail-heavy tree), exported raw to DRAM
  - the host does the final partition-sum + division while un-transposing
    out^T — the device chunk tail is just one psum->sbuf copy + DMA
PE work = QK + PV streaming only ~= compute roofline; ACT exp is the
bottleneck (1 elem/cycle/lane is an ACT hardware floor).
"""

import numpy as np
import ml_dtypes

B = 2
N = 2048
D_MODEL = 2048
H = 16
DH = 128
N_CORES = 8
PAIRS_PER_CORE = (B * H) // N_CORES  # 4
SCALE = float(DH) ** -0.5
CHUNK = 1024  # query chunk (2 psum banks; one key-block row per S tile)
NCHUNKS = N // CHUNK  # 2
QB = 128  # query/key block
NB = N // QB  # 16 key blocks
BLOCKS_PER_CHUNK = CHUNK // QB  # 8
MMN = 512  # max matmul free dim (one psum bank of f32)

_nc_cache = {}
_last_in_maps = None


def _build_nc(reps=0):
    """Build + compile the per-core Bass kernel (same NEFF for all cores).

    reps>0 wraps the body in a dynamic For_i loop running it `reps` times —
    used only for wall-clock benchmarking (the work is idempotent)."""
    from contextlib import ExitStack

    import concourse.bass as bass
    import concourse.mybir as mybir
    import concourse.tile as tile
    from concourse import bacc
    from concourse import bass_isa

    dt_mm = mybir.dt.bfloat16
    f32 = mybir.dt.float32

    nc = bacc.Bacc(
        "TRN2",
        target_bir_lowering=False,
        debug=False,
        enable_asserts=False,
        num_devices=N_CORES,
    )
    P = PAIRS_PER_CORE
    qT_d = nc.dram_tensor("qT", [P, DH, N], dt_mm, kind="ExternalInput").ap()
    kT_d = nc.dram_tensor("kT", [P, DH, N], dt_mm, kind="ExternalInput").ap()
    # v pre-arranged on host to the SBUF layout: [pair, k_local, block*DH+d]
    v_d = nc.dram_tensor("v", [P, QB, NB * DH], dt_mm, kind="ExternalInput").ap()
    outT_d = nc.dram_tensor("outT", [P, DH, N], f32, kind="ExternalOutput").ap()
    # raw softmax-denominator accumulators; the host does the partition-sum
    # and the division (kills the PAR->recip->mul tail chain on device)
    accs_d = nc.dram_tensor(
        "accs", [P, NCHUNKS, QB, CHUNK], dt_mm, kind="ExternalOutput"
    ).ap()

    with tile.TileContext(nc) as tc, ExitStack() as ctx:
        sb = ctx.enter_context(tc.tile_pool(name="sb", bufs=3))
        pt_pool = ctx.enter_context(tc.tile_pool(name="pt", bufs=3))
        acc_pool = ctx.enter_context(tc.tile_pool(name="acc", bufs=2))
        small = ctx.enter_context(tc.tile_pool(name="small", bufs=3))
        outp = ctx.enter_context(tc.tile_pool(name="outp", bufs=2))
        const_pool = ctx.enter_context(tc.tile_pool(name="const", bufs=1))
        ps_s = ctx.enter_context(tc.tile_pool(name="ps_s", bufs=2, space="PSUM"))
        ps_o = ctx.enter_context(tc.tile_pool(name="ps_o", bufs=2, space="PSUM"))

        rep_ctx = ExitStack()
        if reps:
            # hint_engines: body >256 instrs/engine -> back-edge would
            # IRAM-miss (~4us) without prefetch hints; keeps the bench
            # closer to true single-shot time
            rep_ctx.enter_context(
                tc.For_i(
                    0,
                    reps,
                    1,
                    hint_engines=(
                        mybir.EngineType.PE,
                        mybir.EngineType.Activation,
                        mybir.EngineType.DVE,
                        mybir.EngineType.Pool,
                        mybir.EngineType.SP,
                    ),
                )
            )

        def emit_block(st, c, j, jc):
            qoff = max(0, j * QB - c * CHUNK)
            # split [qoff, CHUNK) into <=MMN psum-bank-aligned pieces
            pieces = [(qoff, MMN), (MMN, CHUNK)] if qoff < MMN else [(qoff, CHUNK)]
            s_ps = ps_s.tile([128, CHUNK], f32, tag="s")
            # hoist QKs ahead of the previous block's exp-gated PV cluster
            # (and the previous chunk/pair tail) in the static schedule
            with tc.high_priority(offset=40 if j < 2 else 12):
                for p0, p1 in pieces:
                    nc.tensor.matmul(
                        s_ps[:, p0:p1],
                        lhsT=st["kT"][:, j * QB : (j + 1) * QB],
                        rhs=st["qT"][:, c * CHUNK + p0 : c * CHUNK + p1],
                        start=True,
                        stop=True,
                    )
            pt = st["pt"]
            nc.scalar.activation(
                pt[:, j * CHUNK + qoff : (j + 1) * CHUNK],
                s_ps[:, qoff:],
                mybir.ActivationFunctionType.Exp,
                scale=SCALE,
            )
            if j * QB >= c * CHUNK:  # diagonal block
                # strict-upper triangle of the first 128 valid cols:
                # keep where local_q - local_k >= 0
                nc.gpsimd.affine_select(
                    out=pt[:, j * CHUNK + qoff : j * CHUNK + qoff + QB],
                    in_=pt[:, j * CHUNK + qoff : j * CHUNK + qoff + QB],
                    compare_op=mybir.AluOpType.is_ge,
                    fill=0.0,
                    base=0,
                    channel_multiplier=-1,
                    pattern=[[1, QB]],
                )
            for p0, p1 in pieces:
                nc.tensor.matmul(
                    st["o"][:, p0:p1],
                    lhsT=st["v"][:, j * DH : (j + 1) * DH],
                    rhs=pt[:, j * CHUNK + p0 : j * CHUNK + p1],
                    start=(j == 0),
                    stop=(j == jc - 1),
                )
            # running denominator accumulation (narrowed; spread across
            # the chunk instead of a tail-heavy tree fold)
            if j == 0:
                nc.vector.tensor_copy(st["acc"][:], pt[:, 0:CHUNK])
            else:
                nc.vector.tensor_add(
                    st["acc"][:, qoff:],
                    st["acc"][:, qoff:],
                    pt[:, j * CHUNK + qoff : (j + 1) * CHUNK],
                )

        def emit_tail(st, c):
            # unnormalized out^T to SBUF; acc straight out to DRAM — the
            # host finishes the softmax division
            nc.vector.tensor_copy(
                st["outT"][:, c * CHUNK : (c + 1) * CHUNK], st["o"][:]
            )
            nc.sync.dma_start(accs_d[st["p"], c], st["acc"][:])

        for p in range(P):
            # split input DMAs so the first QK blocks start before the
            # whole pair's operands land
            qT_s = sb.tile([128, N], dt_mm, tag="qT")
            nc.sync.dma_start(qT_s[:, :MMN], qT_d[p][:, :MMN])
            nc.sync.dma_start(qT_s[:, MMN:CHUNK], qT_d[p][:, MMN:CHUNK])
            nc.sync.dma_start(qT_s[:, CHUNK:], qT_d[p][:, CHUNK:])
            kT_s = sb.tile([128, N], dt_mm, tag="kT")
            nc.sync.dma_start(kT_s[:, : 4 * QB], kT_d[p][:, : 4 * QB])
            nc.sync.dma_start(kT_s[:, 4 * QB :], kT_d[p][:, 4 * QB :])
            v_s = sb.tile([128, NB * DH], dt_mm, tag="v")
            nc.sync.dma_start(v_s[:, : 4 * DH], v_d[p][:, : 4 * DH])
            nc.sync.dma_start(v_s[:, 4 * DH :], v_d[p][:, 4 * DH :])
            outT_s = outp.tile([128, N], f32, tag="outT")
            st = {"p": p, "qT": qT_s, "kT": kT_s, "v": v_s, "outT": outT_s}
            for c in range(NCHUNKS):
                jc = BLOCKS_PER_CHUNK * (c + 1)  # key blocks 0..jc-1
                st["pt"] = pt_pool.tile([128, NB * CHUNK], dt_mm, tag="pt", name="pt")
                st["acc"] = acc_pool.tile([128, CHUNK], dt_mm, tag="acc", name="acc")
                st["o"] = ps_o.tile([128, CHUNK], f32, tag="o", name="o")
                for j in range(jc):
                    emit_block(st, c, j, jc)
                emit_tail(st, c)
            nc.sync.dma_start(outT_d[p], outT_s[:])

        rep_ctx.close()

    nc.compile()
    return nc


def _get_nc():
    if "nc" not in _nc_cache:
        _nc_cache["nc"] = _build_nc()
    return _nc_cache["nc"]


def kernel(q, k, v):
    from concourse.bass_utils import run_bass_kernel_spmd

    q = np.asarray(q, dtype=np.float32)
    k = np.asarray(k, dtype=np.float32)
    v = np.asarray(v, dtype=np.float32)

    bf16 = ml_dtypes.bfloat16
    # [b, n, h, dh] -> [b, h, dh, n] for q/k; [b, h, n, dh] for v
    qT = np.ascontiguousarray(
        q.reshape(B, N, H, DH).transpose(0, 2, 3, 1)
    ).astype(bf16)
    kT = np.ascontiguousarray(
        k.reshape(B, N, H, DH).transpose(0, 2, 3, 1)
    ).astype(bf16)
    # v -> [b, h, k_local, block, dh]: v_host[p, k, j*DH+d] = v[p, j*QB+k, d]
    vh = np.ascontiguousarray(
        v.reshape(B, NB, QB, H, DH).transpose(0, 3, 2, 1, 4)
    ).astype(bf16)

    qT = qT.reshape(B * H, DH, N)
    kT = kT.reshape(B * H, DH, N)
    vh = vh.reshape(B * H, QB, NB * DH)

    in_maps = []
    for core in range(N_CORES):
        lo = core * PAIRS_PER_CORE
        hi = lo + PAIRS_PER_CORE
        in_maps.append(
            {
                "qT": np.ascontiguousarray(qT[lo:hi]),
                "kT": np.ascontiguousarray(kT[lo:hi]),
                "v": np.ascontiguousarray(vh[lo:hi]),
            }
        )

    global _last_in_maps
    _last_in_maps = in_maps

    nc = _get_nc()
    res = run_bass_kernel_spmd(nc, in_maps, core_ids=list(range(N_CORES)))

    # reassemble: outT per core [P, dh, n] f32 (unnormalized) -> normalize by
    # the softmax denominators (sum the raw accumulators over partitions),
    # then back to [b, n, h*dh]
    outT = np.concatenate([r["outT"] for r in res.results], axis=0)  # [32, dh, n]
    accs = np.concatenate([r["accs"] for r in res.results], axis=0)
    # accs: [32, NCHUNKS, 128, CHUNK] bf16 -> sums [32, n]
    sums = accs.astype(np.float32).sum(axis=2).reshape(B * H, N)
    outT = outT / sums[:, None, :]
    out = outT.reshape(B, H, DH, N).transpose(0, 3, 1, 2).reshape(B, N, D_MODEL)
    return np.ascontiguousarray(out)
